# revision 1
# baseline (speedup 1.0000x reference)
"""Trainium2 Bass kernel for nn_LRSVConv (low-rank spatially-varying conv).

Computes, for full inputs
    x            [8, 32, 256, 256]  f32
    conv_w       [192, 32, 3, 3]    f32   (192 = RANK(3) * C_OUT(64))
    kernel_weight[2, 256, 256]      f32
the reference:
    y   = conv2d(x, conv_w, stride 1, pad 1)      # [8, 192, 256, 256]
    y   = y.reshape(8, 3, 64, 256, 256)
    out = y[:,0] + kw[0]*y[:,1] + kw[1]*y[:,2]    # [8, 64, 256, 256]

Strategy: spatial (H) sharding across 8 cores - each core computes a band of
32 output rows for ALL batches, so the per-pixel blend weights (which are
batch-independent) are loaded/broadcast once per core and reused 8x.

Per core:
  - imcol tile [96, 32*258]: 3 kh-shifted replicas of the padded input rows
    (partition dim = (kh, c_in)), padded W=258 so kw shifts are free-dim
    offsets and no edge handling is needed.
  - conv: per supertile (4 output rows = 1024 px, split into 2 blocks of
    512 px), per rank r and kw: one K=96, M=64, N=512 fp32 matmul per block,
    the two blocks on opposite column halves of the PE array (concurrent via
    col tiling), accumulating in PSUM banks A/B/C (one per rank); psum rows
    = (block, c_out).
  - blend: t1 = B * sv1_bcast, t2 = C * sv2_bcast on DVE; t1 accumulated
    onto A via an identity matmul on the (otherwise busier) TensorE;
    out = A + t2 on DVE (fused PSUM evacuation).
  - sv broadcast tiles are prepared host-side ([128, 4096] per rank: rows
    (block, c) x band pixels) - tiny input, avoids on-device partition
    broadcast which no engine does well.
"""

import os

import numpy as np

B, C_IN, C_OUT, RANK, IMG = 8, 32, 64, 3, 256
N_CORES = 8
BAND = IMG // N_CORES          # 32 output rows per core
WP = IMG + 2                   # padded width 258
ROWS_IN = BAND + 2             # input rows needed per band (with halo)
SUPER = 8                      # supertiles per (batch, band): 4 rows each
SROWS = BAND // SUPER          # 4 image rows per supertile
NBLK = 512                     # pixels per matmul block (2 image rows)

_F32 = np.float32

# "pe": rank-1 partial added into PSUM A by an identity matmul on TensorE
# "dve": both adds on VectorE (simpler, more DVE load)
BLEND_MODE = os.environ.get("KERNEL_BLEND", "pe")
NB = int(os.environ.get("KERNEL_NB", str(B)))  # batches to process (debug knob)


def _build_bass():
    import concourse.mybir as mybir
    import concourse.tile as tile
    from concourse import bacc

    f32 = mybir.dt.float32
    # float32r: single-pass PE fp32 (1 cyc/row at N>=256 vs 4 for fp32)
    f32r = mybir.dt.float32r
    nc = bacc.Bacc("TRN2", target_bir_lowering=False, debug=False)

    xs_t = nc.dram_tensor("xs", (B, C_IN, ROWS_IN * WP), f32r, kind="ExternalInput")
    # wtBC[kw]: [96, (rank1|rank2)]; wtA[kw, q]: [96, (w0|0) or (0|w0)]
    wtbc_t = nc.dram_tensor("wtbc", (3, 96, 128), f32r, kind="ExternalInput")
    wta_t = nc.dram_tensor("wta", (3, 2, 96, 128), f32r, kind="ExternalInput")
    # S12: rows 0:64 = sv1, rows 64:128 = sv2; cols = (supertile, block, j)
    svb_t = nc.dram_tensor("svb", (128, SUPER * 2 * NBLK), f32, kind="ExternalInput")
    # identII[q]: cols 64q:64q+64 hold [I64; I64] (sum the two 64-row halves)
    id_t = nc.dram_tensor("ident", (2, 128, 128), f32r, kind="ExternalInput")
    out_t = nc.dram_tensor("out", (B, C_OUT, BAND, IMG), f32, kind="ExternalOutput")

    xs = xs_t.ap()
    out_r = out_t.ap().rearrange(
        "b c (t q r) w -> b q c t (r w)", t=SUPER, q=2, r=SROWS // 2
    )

    with tile.TileContext(nc) as tc:
        with (
            tc.tile_pool(name="const", bufs=1) as cpool,
            tc.tile_pool(name="imcol", bufs=2) as ipool,
            tc.tile_pool(name="psum", bufs=2, space="PSUM") as ppool,
            tc.tile_pool(name="tmp", bufs=3) as tpool,
            tc.tile_pool(name="outp", bufs=4) as opool,
        ):
            wtbc_sb = cpool.tile([96, 3, 128], f32r)
            nc.sync.dma_start(wtbc_sb[:], wtbc_t.ap().rearrange("k p m -> p k m"))
            wta_sb = cpool.tile([96, 3, 2, 128], f32r)
            nc.sync.dma_start(wta_sb[:], wta_t.ap().rearrange("k q p m -> p k q m"))
            svb_sb = cpool.tile([128, SUPER * 2 * NBLK], f32)
            nc.sync.dma_start(svb_sb[:], svb_t.ap())
            id_sb = cpool.tile([128, 2, 128], f32r)
            nc.sync.dma_start(id_sb[:], id_t.ap().rearrange("q p m -> p q m"))

            for b in range(NB):
                imcol = ipool.tile([96, BAND * WP], f32r, tag="imcol")
                for kh in range(3):
                    nc.sync.dma_start(
                        imcol[32 * kh : 32 * kh + 32, :],
                        xs[b, :, kh * WP : kh * WP + BAND * WP],
                    )
                imv = imcol.rearrange("p (h w) -> p h w", w=WP)

                for t in range(SUPER):
                    bc = ppool.tile([128, 2 * NBLK], f32, tag="bc")
                    a2 = ppool.tile([128, NBLK], f32, tag="a2")
                    for kw in range(3):
                        for q in range(2):
                            hl = SROWS * t + 2 * q
                            rhs = imv[:, hl : hl + 2, kw : kw + IMG]
                            nc.tensor.matmul(
                                bc[:, NBLK * q : NBLK * (q + 1)],
                                wtbc_sb[:, kw, :],
                                rhs,
                                start=(kw == 0),
                                stop=(kw == 2),
                            )
                            nc.tensor.matmul(
                                a2[:],
                                wta_sb[:, kw, q, :],
                                rhs,
                                start=(kw == 0 and q == 0),
                                stop=False,
                            )

                    # m = [sv1*y1 ; sv2*y2] for both blocks, one 128-row op
                    m = tpool.tile([128, 2 * NBLK], f32r, tag="m")
                    nc.vector.tensor_tensor(
                        m[:],
                        bc,
                        svb_sb[:, 2 * NBLK * t : 2 * NBLK * (t + 1)],
                        mybir.AluOpType.mult,
                    )
                    # fold the two 64-row halves of m into a2 rows (q*64..)
                    for q in range(2):
                        nc.tensor.matmul(
                            a2[:],
                            id_sb[:, q, :],
                            m[:, NBLK * q : NBLK * (q + 1)],
                            start=False,
                            stop=(q == 1),
                        )
                    out_sb = opool.tile([128, NBLK], f32, tag="out_sb")
                    nc.scalar.copy(out_sb[:], a2[:])
                    for q in range(2):
                        nc.sync.dma_start(
                            out_r[b, q, :, t, :], out_sb[64 * q : 64 * q + 64, :]
                        )
    nc.compile()
    return nc


_CACHE = {}


def _get_bass():
    if "nc" not in _CACHE:
        _CACHE["nc"] = _build_bass()
    return _CACHE["nc"]


def _prep_shards(x, conv_w, kernel_weight):
    x = np.asarray(x, dtype=_F32)
    conv_w = np.asarray(conv_w, dtype=_F32)
    kernel_weight = np.asarray(kernel_weight, dtype=_F32)

    x_pad = np.pad(x, ((0, 0), (0, 0), (1, 1), (1, 1)))
    # w[kh, c, kw, (r, m)] from conv_w[(r m), c, kh, kw]
    wt = conv_w.transpose(2, 1, 3, 0).reshape(96, 3, RANK * C_OUT)
    wtbc = np.ascontiguousarray(
        wt[:, :, C_OUT:].reshape(96, 3, 128).transpose(1, 0, 2)
    )  # [kw, 96, (r1|r2)]
    wta = np.zeros((3, 2, 96, 128), dtype=_F32)
    for q in range(2):
        wta[:, q, :, 64 * q : 64 * q + 64] = wt[:, :, :C_OUT].transpose(1, 0, 2)
    ident = np.zeros((2, 128, 128), dtype=_F32)
    for q in range(2):
        ident[q, 0:64, 64 * q : 64 * q + 64] = np.eye(64, dtype=_F32)
        ident[q, 64:128, 64 * q : 64 * q + 64] = np.eye(64, dtype=_F32)

    in_maps = []
    for i in range(N_CORES):
        h0 = BAND * i
        shard = np.ascontiguousarray(
            x_pad[:, :, h0 : h0 + ROWS_IN, :]
        ).reshape(B, C_IN, ROWS_IN * WP)
        band = kernel_weight[:, h0 : h0 + BAND, :]          # [2, 32, 256]
        # svb[64r+c, (t, q, j)] = band[r, row(t, q, j)]
        arr = band.reshape(2, SUPER, 2 * NBLK)              # [r, t, (q j)]
        svb = np.broadcast_to(
            arr[:, None, :, :], (2, C_OUT, SUPER, 2 * NBLK)
        ).reshape(128, SUPER * 2 * NBLK)
        svb = np.ascontiguousarray(svb)
        in_maps.append(
            {"xs": shard, "wtbc": wtbc, "wta": wta, "svb": svb, "ident": ident}
        )
    return in_maps


def run(inputs, trace=False):
    """Run the sharded bass kernel; returns (out_full, BassKernelResults)."""
    from concourse.bass_utils import run_bass_kernel_spmd

    in_maps = _prep_shards(**inputs)
    nc = _get_bass()
    res = run_bass_kernel_spmd(
        nc, in_maps, core_ids=list(range(N_CORES)), trace=trace
    )
    out = np.empty((B, C_OUT, IMG, IMG), dtype=_F32)
    for i in range(N_CORES):
        out[:, :, BAND * i : BAND * (i + 1), :] = res.results[i]["out"]
    return out, res


def kernel(x, conv_w, kernel_weight):
    out, _ = run({"x": x, "conv_w": conv_w, "kernel_weight": kernel_weight})
    return out



# revision 8
# speedup vs baseline: 1.5989x; 1.5989x over previous
"""Trainium2 Bass kernel for nn_LRSVConv (low-rank spatially-varying conv).

Computes, for full inputs
    x            [8, 32, 256, 256]  f32
    conv_w       [192, 32, 3, 3]    f32   (192 = RANK(3) * C_OUT(64))
    kernel_weight[2, 256, 256]      f32
the reference:
    y   = conv2d(x, conv_w, stride 1, pad 1)      # [8, 192, 256, 256]
    y   = y.reshape(8, 3, 64, 256, 256)
    out = y[:,0] + kw[0]*y[:,1] + kw[1]*y[:,2]    # [8, 64, 256, 256]

Sharding: spatial (H) bands of 32 output rows across the 8 cores, all
batches per core, so the per-pixel blend weights are loaded once per core.

Per core, per supertile (4 output rows = 2 blocks q of 512 px):
  - conv matmuls in bf16 (FWL-able weight loads, exact-enough: measured
    rel err 2.4e-3 vs the 2e-2 gate), K=96 (3 kh x 32 c_in), kw via
    free-dim shifts of a 3x-replicated imcol tile:
      per kw: bc[q0] (M=128: ranks 1|2), bc[q1], and a col-tiled
      concurrent pair a2[0:64] / a2[64:128] (M=64: rank 0, blocks q0/q1)
    -> 9 matmul time-slots instead of 12.
  - blend: m = bc * svb on DVE (svb = host-broadcast sv weights), then a
    col-tiled pair of identity matmuls folds m's two 64-row halves into
    a2 -> out = y0 + sv1*y1 + sv2*y2 in PSUM; ACT evacuates to SBUF.
  - blend stage runs one supertile behind the conv stage so the PE never
    waits for DVE.
  - imcol for batch b+1 is prefetched (single 3-window DMA) at the start
    of batch b so batch boundaries don't stall the PE.
"""

import numpy as np
import ml_dtypes

B, C_IN, C_OUT, RANK, IMG = 8, 32, 64, 3, 256
N_CORES = 8
BAND = IMG // N_CORES          # 32 output rows per core
WP = IMG + 2                   # padded width 258
ROWS_IN = BAND + 2             # input rows needed per band (with halo)
SUPER = 8                      # supertiles per (batch, band): 4 rows each
SROWS = BAND // SUPER          # 4 image rows per supertile
NBLK = 512                     # pixels per matmul block (2 image rows)

_F32 = np.float32
_BF16 = ml_dtypes.bfloat16


def _build_bass():
    import concourse.mybir as mybir
    import concourse.tile as tile
    from concourse import bacc

    f32 = mybir.dt.float32
    f32r = mybir.dt.float32r
    bf16 = mybir.dt.bfloat16
    nc = bacc.Bacc("TRN2", target_bir_lowering=False, debug=False)

    xs_t = nc.dram_tensor("xs", (B, C_IN, ROWS_IN * WP), bf16, kind="ExternalInput")
    # wtbc[kw]: [96, 128] ranks (1|2); wta[kw]: [96, 64] rank 0
    wtbc_t = nc.dram_tensor("wtbc", (3, 96, 128), bf16, kind="ExternalInput")
    wta_t = nc.dram_tensor("wta", (3, 96, 64), bf16, kind="ExternalInput")
    # svb rows 0:64 = sv1 (bcast over c), rows 64:128 = sv2; cols = (t, q, j)
    svb_t = nc.dram_tensor("svb", (128, SUPER * 2 * NBLK), bf16, kind="ExternalInput")
    # ident: [128, 64] = [I64; I64] (fold the two 64-row halves)
    id_t = nc.dram_tensor("ident", (128, 64), bf16, kind="ExternalInput")
    out_t = nc.dram_tensor("out", (B, C_OUT, BAND, IMG), f32, kind="ExternalOutput")

    out_r = out_t.ap().rearrange(
        "b c (t q r) w -> b q c t (r w)", t=SUPER, q=2, r=SROWS // 2
    )

    with tile.TileContext(nc) as tc:
        with (
            tc.tile_pool(name="const", bufs=1) as cpool,
            tc.tile_pool(name="imcol", bufs=3) as ipool,
            tc.tile_pool(name="psum", bufs=2, space="PSUM") as ppool,
            tc.tile_pool(name="tmp", bufs=3) as tpool,
            tc.tile_pool(name="outp", bufs=4) as opool,
        ):
            wtbc_sb = cpool.tile([96, 3, 128], bf16)
            nc.sync.dma_start(wtbc_sb[:], wtbc_t.ap().rearrange("k p m -> p k m"))
            wta_sb = cpool.tile([96, 3, 64], bf16)
            nc.sync.dma_start(wta_sb[:], wta_t.ap().rearrange("k p m -> p k m"))
            svb_sb = cpool.tile([128, SUPER, 2 * NBLK], bf16)
            nc.sync.dma_start(
                svb_sb[:], svb_t.ap().rearrange("p (t j) -> p t j", t=SUPER)
            )
            id_sb = cpool.tile([128, 64], bf16)
            nc.sync.dma_start(id_sb[:], id_t.ap())

            # Flat software pipeline over (b, t): conv stage at i, blend
            # stage at i-1 so the PE never waits on DVE.
            steps = [(b, t) for b in range(B) for t in range(SUPER)]
            conv_state = {}  # i -> (bc, a2)
            imcols = {}

            for i, (b, t) in enumerate(steps):
                if t == 0:
                    # prefetch next batch's imcol (current batch's was
                    # prefetched one batch ago; b==0 loads immediately)
                    if b == 0:
                        imc = ipool.tile([96, BAND * WP], bf16, tag="imcol")
                        for kh in range(3):
                            nc.sync.dma_start(
                                imc[32 * kh : 32 * kh + 32, :],
                                xs_t.ap()[b, :, kh * WP : kh * WP + BAND * WP],
                            )
                        imcols[0] = imc
                    if b + 1 < B:
                        imc = ipool.tile([96, BAND * WP], bf16, tag="imcol")
                        for kh in range(3):
                            nc.sync.dma_start(
                                imc[32 * kh : 32 * kh + 32, :],
                                xs_t.ap()[b + 1, :, kh * WP : kh * WP + BAND * WP],
                            )
                        imcols[b + 1] = imc

                imv = imcols[b].rearrange("p (h w) -> p h w", w=WP)

                # ---- conv stage for step i ----
                bc = ppool.tile([128, 2, NBLK], f32, tag="bc")
                a2 = ppool.tile([128, NBLK], f32, tag="a2")
                hl = SROWS * t
                for kw in range(3):
                    rhs0 = imv[:, hl : hl + 2, kw : kw + IMG]
                    rhs1 = imv[:, hl + 2 : hl + 4, kw : kw + IMG]
                    nc.tensor.matmul(
                        bc[:, 0, :], wtbc_sb[:, kw, :], rhs0,
                        start=(kw == 0), stop=(kw == 2),
                    )
                    nc.tensor.matmul(
                        bc[:, 1, :], wtbc_sb[:, kw, :], rhs1,
                        start=(kw == 0), stop=(kw == 2),
                    )
                    # rank-0 for both blocks: col-tiled concurrent pair
                    nc.tensor.matmul(
                        a2[0:64, :], wta_sb[:, kw, :], rhs0,
                        start=(kw == 0), stop=False, skip_group_check=True,
                    )
                    nc.tensor.matmul(
                        a2[64:128, :], wta_sb[:, kw, :], rhs1,
                        start=(kw == 0), stop=False, skip_group_check=True,
                    )
                conv_state[i] = (bc, a2)

                # ---- blend stage for step i-1 ----
                if i >= 1:
                    _blend(nc, tc, tpool, opool, conv_state, i - 1, steps,
                           svb_sb, id_sb, out_r, f32, f32r)
                    del conv_state[i - 1]

            _blend(nc, tc, tpool, opool, conv_state, len(steps) - 1, steps,
                   svb_sb, id_sb, out_r, f32, f32r)

    nc.compile()
    return nc


def _blend(nc, tc, tpool, opool, conv_state, i, steps, svb_sb, id_sb, out_r,
           f32, f32r):
    import concourse.mybir as mybir

    bf16 = mybir.dt.bfloat16
    b, t = steps[i]
    bc, a2 = conv_state[i]
    m = tpool.tile([128, 2, NBLK], bf16, tag="m")
    nc.vector.tensor_tensor(
        m[:], bc[:], svb_sb[:, t, :].rearrange("p (q j) -> p q j", q=2),
        mybir.AluOpType.mult,
    )
    # fold m's halves into a2: col-tiled concurrent pair
    nc.tensor.matmul(
        a2[0:64, :], id_sb[:], m[:, 0, :],
        start=False, stop=False, skip_group_check=True,
    )
    nc.tensor.matmul(
        a2[64:128, :], id_sb[:], m[:, 1, :],
        start=False, stop=True, skip_group_check=True,
    )
    out_sb = opool.tile([128, NBLK], f32, tag="out_sb")
    nc.scalar.copy(out_sb[:], a2[:])
    for q in range(2):
        nc.sync.dma_start(out_r[b, q, :, t, :], out_sb[64 * q : 64 * q + 64, :])


_CACHE = {}


def _get_bass():
    if "nc" not in _CACHE:
        _CACHE["nc"] = _build_bass()
    return _CACHE["nc"]


def _prep_shards(x, conv_w, kernel_weight):
    x = np.asarray(x, dtype=_F32)
    conv_w = np.asarray(conv_w, dtype=_F32)
    kernel_weight = np.asarray(kernel_weight, dtype=_F32)

    x_pad = np.pad(x, ((0, 0), (0, 0), (1, 1), (1, 1)))
    # w[kh, c, kw, (r, m)] from conv_w[(r m), c, kh, kw]
    wt = conv_w.transpose(2, 1, 3, 0).reshape(96, 3, RANK * C_OUT)
    wtbc = np.ascontiguousarray(wt[:, :, C_OUT:].transpose(1, 0, 2)).astype(_BF16)
    wta = np.ascontiguousarray(wt[:, :, :C_OUT].transpose(1, 0, 2)).astype(_BF16)
    ident = np.concatenate(
        [np.eye(64, dtype=_F32), np.eye(64, dtype=_F32)], axis=0
    ).astype(_BF16)

    in_maps = []
    for i in range(N_CORES):
        h0 = BAND * i
        shard = (
            np.ascontiguousarray(x_pad[:, :, h0 : h0 + ROWS_IN, :])
            .reshape(B, C_IN, ROWS_IN * WP)
            .astype(_BF16)
        )
        band = kernel_weight[:, h0 : h0 + BAND, :]          # [2, 32, 256]
        # svb[64r+c, (t, q, j)] = band[r, row(t, q, j)]
        arr = band.reshape(2, SUPER, 2 * NBLK)              # [r, t, (q j)]
        svb = np.broadcast_to(
            arr[:, None, :, :], (2, C_OUT, SUPER, 2 * NBLK)
        ).reshape(128, SUPER * 2 * NBLK)
        svb = np.ascontiguousarray(svb).astype(_BF16)
        in_maps.append(
            {"xs": shard, "wtbc": wtbc, "wta": wta, "svb": svb, "ident": ident}
        )
    return in_maps


def run(inputs, trace=False):
    """Run the sharded bass kernel; returns (out_full, BassKernelResults)."""
    from concourse.bass_utils import run_bass_kernel_spmd

    in_maps = _prep_shards(**inputs)
    nc = _get_bass()
    res = run_bass_kernel_spmd(
        nc, in_maps, core_ids=list(range(N_CORES)), trace=trace
    )
    out = np.empty((B, C_OUT, IMG, IMG), dtype=_F32)
    for i in range(N_CORES):
        out[:, :, BAND * i : BAND * (i + 1), :] = res.results[i]["out"]
    return out, res


def kernel(x, conv_w, kernel_weight):
    out, _ = run({"x": x, "conv_w": conv_w, "kernel_weight": kernel_weight})
    return out
